# revision 1
# baseline (speedup 1.0000x reference)
"""Paged KV-cache decode attention with ALiBi (Baichuan-style), fused
QKV + attention + output projection, tensor-parallel over heads across
8 Trainium2 NeuronCores.

Layout strategy (per core, 5 heads):
  - qT/kT computed as [640, 4] (head-dim on partitions) so scores matmuls
    need no transposes and the K-cache new-token scatter is a same-partition
    SBUF copy.
  - v computed as [4, 640] (natural) so the V new-token scatter is a tiny
    SBUF->SBUF DMA row write.
  - K cache staged host-side per core as [5, 4, 128(d), 2048(t)] (K^T),
    V cache as [5, 4, 128(t%128), 16(chunk), 128(d)] so every device DMA is
    a large (>=0.5-1MB) mostly-contiguous transfer.
  - softmax without max-subtraction (scores are O(10); exp is safe in fp32),
    masking baked into a host-precomputed additive bias (-1e30).
  - o_proj computed transposed (out^T [5120, 4]) per core; host sums the 8
    partial products (the "all-reduce").
"""

import math
import os
import sys
from contextlib import ExitStack

import numpy as np

sys.path.insert(0, "/opt/trn_rl_repo")

B = 4
E = 5120
H = 40
D = 128
BS = 16
NB = 512
MB = 128
S = MB * BS  # 2048
NCORES = 8
HPC = H // NCORES   # 5 heads per core
EPC = HPC * D       # 640

NEG = -1.0e30


def _alibi_slopes(num_heads):
    cp2 = 2 ** int(math.floor(math.log2(num_heads)))
    base = 2.0 ** (-(2.0 ** (-(math.log2(cp2) - 3))))
    slopes = base ** np.arange(1, cp2 + 1, dtype=np.float64)
    if cp2 != num_heads:
        extra_base = 2.0 ** (-(2.0 ** (-(math.log2(2 * cp2) - 3))))
        n_rem = min(cp2, num_heads - cp2)
        extra = extra_base ** np.arange(1, 1 + 2 * n_rem, 2, dtype=np.float64)
        slopes = np.concatenate([slopes, extra])
    return slopes.astype(np.float32)


_PROGRAM_CACHE = {}
LAST_RESULTS = None  # BassKernelResults of the most recent run (for test.py)


def _build_program(pos, nch):
    """Build the SPMD Bass program. pos/nch are per-sequence tuples, baked
    statically (same for all cores; per-core data varies only via inputs)."""
    import concourse.bacc as bacc
    import concourse.bass as bass
    import concourse.tile as tile
    from concourse import mybir

    f32 = mybir.dt.float32
    nc = bacc.Bacc()

    hT = nc.declare_dram_parameter("hT", [128, 40 * B], f32, isOutput=False)
    qkvw = nc.declare_dram_parameter("qkvw", [3, E, EPC], f32, isOutput=False)
    ow = nc.declare_dram_parameter("ow", [EPC, E], f32, isOutput=False)
    kt = nc.declare_dram_parameter("kt", [HPC, B, D, S], f32, isOutput=False)
    vt = nc.declare_dram_parameter("vt", [HPC, B, 128, 16, D], f32, isOutput=False)
    bias = nc.declare_dram_parameter("bias", [128, B * HPC * 16], f32, isOutput=False)
    outT = nc.declare_dram_parameter("outT", [128, 40 * B], f32, isOutput=True)

    with tile.TileContext(nc) as tc, ExitStack() as ctx:
        consts = ctx.enter_context(tc.tile_pool(name="consts", bufs=1))
        wpool = ctx.enter_context(tc.tile_pool(name="wpool", bufs=2))
        kvpool = ctx.enter_context(tc.tile_pool(name="kvpool", bufs=3))
        tmp = ctx.enter_context(tc.tile_pool(name="tmp", bufs=3))
        opool = ctx.enter_context(tc.tile_pool(name="opool", bufs=2))
        psum = ctx.enter_context(tc.tile_pool(name="psum", bufs=8, space="PSUM"))

        # ---- constants / small inputs ----
        hT_sb = consts.tile([128, 40 * B], f32)          # (E%128, (Echunk, b))
        nc.gpsimd.dma_start(out=hT_sb[:], in_=hT[:])
        bias_sb = consts.tile([128, B * HPC * 16], f32)  # (t%128, (b, h, chunk))
        nc.gpsimd.dma_start(out=bias_sb[:], in_=bias[:])
        ones_col = consts.tile([128, 1], f32)
        nc.vector.memset(ones_col[:], 1.0)
        ones_row = consts.tile([1, 128], f32)
        nc.vector.memset(ones_row[:], 1.0)

        qT_sb = consts.tile([128, HPC * B], f32)   # col = h*B + b ; partition = d
        kT_sb = consts.tile([128, HPC * B], f32)
        v_sb = consts.tile([B, EPC], f32)          # natural v rows
        colsum_sb = consts.tile([128, HPC * B], f32)
        aoT_sb = consts.tile([128, HPC * B], f32)  # unnormalized attn@V ^T
        outT_sb = consts.tile([128, 40 * B], f32)

        # ---- fused QKV projection ----
        # q,k transposed orientation: psum[oc] [128, B] accumulated over 40
        # E-chunks; lhsT = W chunk [128(E), 128(outcol)], rhs = hT chunk [128(E), B].
        for w in range(2):  # 0=q (pre-scaled on host), 1=k
            dst = qT_sb if w == 0 else kT_sb
            ps = [psum.tile([128, B], f32, tag="ps", name=f"ps_qk{w}_{i}") for i in range(HPC)]
            for g in range(10):  # groups of 4 E-chunks
                wt = wpool.tile([128, 4 * EPC], f32, tag="w")
                nc.gpsimd.dma_start(
                    out=wt[:],
                    in_=qkvw[w, g * 512:(g + 1) * 512, :].rearrange(
                        "(kl p) c -> p kl c", p=128
                    ),
                )
                for oc in range(HPC):
                    for kl in range(4):
                        kc = g * 4 + kl
                        nc.tensor.matmul(
                            ps[oc][:],
                            lhsT=wt[:, kl * EPC + oc * 128: kl * EPC + (oc + 1) * 128],
                            rhs=hT_sb[:, kc * B:(kc + 1) * B],
                            start=(kc == 0),
                            stop=(kc == 39),
                        )
            for oc in range(HPC):
                nc.scalar.copy(dst[:, oc * B:(oc + 1) * B], ps[oc][:])

        # v natural orientation: psum [B, 640] (two banks: 512 + 128),
        # lhsT = hT chunk [128(E), B], rhs = Wv chunk [128(E), 640].
        v_ps0 = psum.tile([B, 512], f32, tag="ps")
        v_ps1 = psum.tile([B, EPC - 512], f32, tag="ps")
        for g in range(10):
            wt = wpool.tile([128, 4 * EPC], f32, tag="w")
            nc.gpsimd.dma_start(
                out=wt[:],
                in_=qkvw[2, g * 512:(g + 1) * 512, :].rearrange(
                    "(kl p) c -> p kl c", p=128
                ),
            )
            for kl in range(4):
                kc = g * 4 + kl
                nc.tensor.matmul(
                    v_ps0[:],
                    lhsT=hT_sb[:, kc * B:(kc + 1) * B],
                    rhs=wt[:, kl * EPC: kl * EPC + 512],
                    start=(kc == 0),
                    stop=(kc == 39),
                )
                nc.tensor.matmul(
                    v_ps1[:],
                    lhsT=hT_sb[:, kc * B:(kc + 1) * B],
                    rhs=wt[:, kl * EPC + 512: kl * EPC + EPC],
                    start=(kc == 0),
                    stop=(kc == 39),
                )
        nc.scalar.copy(v_sb[:, :512], v_ps0[:])
        nc.scalar.copy(v_sb[:, 512:], v_ps1[:])

        # ---- attention per (b, h) ----
        for b in range(B):
            n = nch[b]
            sd = n * 128
            p = pos[b]
            for h in range(HPC):
                col = h * B + b
                Kt = kvpool.tile([128, S], f32, tag="K")
                nc.gpsimd.dma_start(out=Kt[:, :sd], in_=kt[h, b, :, :sd])
                Vt = kvpool.tile([128, 16, D], f32, tag="V")
                nc.gpsimd.dma_start(out=Vt[:, :n, :], in_=vt[h, b, :, :n, :])

                # scatter the new token K column (same partitions: d)
                nc.vector.tensor_copy(Kt[:, p:p + 1], kT_sb[:, col:col + 1])
                # scatter the new token V row (cross-partition -> DMA)
                nc.gpsimd.dma_start(
                    out=Vt[p % 128:p % 128 + 1, p // 128, :],
                    in_=v_sb[b:b + 1, h * D:(h + 1) * D],
                )

                sc_ps = psum.tile([128, 16], f32, tag="ps")
                for c in range(n):
                    nc.tensor.matmul(
                        sc_ps[:, c:c + 1],
                        lhsT=Kt[:, c * 128:(c + 1) * 128],
                        rhs=qT_sb[:, col:col + 1],
                        start=True,
                        stop=True,
                    )
                s_sb = tmp.tile([128, 16], f32, tag="s")
                nc.vector.tensor_add(
                    s_sb[:, :n],
                    sc_ps[:, :n],
                    bias_sb[:, (b * HPC + h) * 16:(b * HPC + h) * 16 + n],
                )
                attn_sb = tmp.tile([128, 16], f32, tag="attn")
                nc.scalar.activation(
                    attn_sb[:, :n],
                    s_sb[:, :n],
                    func=mybir.ActivationFunctionType.Exp,
                    accum_out=colsum_sb[:, col:col + 1],
                )
                ao_ps = psum.tile([128, 1], f32, tag="ps")
                for c in range(n):
                    nc.tensor.matmul(
                        ao_ps[:],
                        lhsT=Vt[:, c, :],
                        rhs=attn_sb[:, c:c + 1],
                        start=(c == 0),
                        stop=(c == n - 1),
                    )
                nc.scalar.copy(aoT_sb[:, col:col + 1], ao_ps[:])

        # ---- softmax normalization (batched over all 20 (b,h)) ----
        sums_ps = psum.tile([1, HPC * B], f32, tag="ps")
        nc.tensor.matmul(
            sums_ps[:], lhsT=ones_col[:], rhs=colsum_sb[:], start=True, stop=True
        )
        recip_sb = tmp.tile([1, HPC * B], f32, tag="recip")
        nc.vector.reciprocal(recip_sb[:], sums_ps[:])
        rb_ps = psum.tile([128, HPC * B], f32, tag="ps")
        nc.tensor.matmul(
            rb_ps[:], lhsT=ones_row[:], rhs=recip_sb[:], start=True, stop=True
        )
        recip_b = tmp.tile([128, HPC * B], f32, tag="recipb")
        nc.vector.tensor_copy(recip_b[:], rb_ps[:])
        attn_nT = consts.tile([128, HPC * B], f32)
        nc.vector.tensor_mul(attn_nT[:], aoT_sb[:], recip_b[:])

        # ---- output projection (transposed): outT[oc*128+p, b] ----
        # lhsT = o chunk [128(hd), 128(oc)], rhs = attn_nT slice [128(hd), B]
        for jg in range(5):  # groups of 8 outcol chunks (1024 cols)
            ops = [psum.tile([128, B], f32, tag="ps", name=f"ps_o{jg}_{i}") for i in range(8)]
            for h in range(HPC):
                ot = opool.tile([128, 1024], f32, tag="ot")
                nc.gpsimd.dma_start(
                    out=ot[:],
                    in_=ow[h * 128:(h + 1) * 128, jg * 1024:(jg + 1) * 1024],
                )
                for oc in range(8):
                    nc.tensor.matmul(
                        ops[oc][:],
                        lhsT=ot[:, oc * 128:(oc + 1) * 128],
                        rhs=attn_nT[:, h * B:(h + 1) * B],
                        start=(h == 0),
                        stop=(h == HPC - 1),
                    )
            for oc in range(8):
                g_oc = jg * 8 + oc
                nc.scalar.copy(outT_sb[:, g_oc * B:(g_oc + 1) * B], ops[oc][:])

        nc.gpsimd.dma_start(out=outT[:], in_=outT_sb[:])

    nc.compile()  # Bacc finalize: splits multi-waits (matmul 1-wait limit)
    return nc


def _prepare_core_inputs(core, hidden, qkv_w, o_w, k_cache, v_cache, bt, sl, pos):
    hs = slice(core * HPC, (core + 1) * HPC)
    es = slice(core * EPC, (core + 1) * EPC)

    qkvw = np.ascontiguousarray(qkv_w[:, :, es])
    qkvw[0] *= np.float32(D ** -0.5)

    kg = k_cache[:, hs]  # [NB, HPC, BS, D]
    vg = v_cache[:, hs]
    kt = np.empty((HPC, B, D, S), np.float32)
    vt = np.empty((HPC, B, 128, 16, D), np.float32)
    for b in range(B):
        kk = kg[bt[b]].transpose(1, 0, 2, 3).reshape(HPC, S, D)
        kt[:, b] = kk.transpose(0, 2, 1)
        vv = vg[bt[b]].transpose(1, 0, 2, 3).reshape(HPC, S, D)
        vt[:, b] = vv.reshape(HPC, 16, 128, D).transpose(0, 2, 1, 3)

    slopes = _alibi_slopes(H)[core * HPC:(core + 1) * HPC]
    t_in = np.arange(128)[:, None]
    tg = (np.arange(16)[None, :] * 128 + t_in).astype(np.float32)  # [128, 16]
    bias = np.empty((128, B, HPC, 16), np.float32)
    for b in range(B):
        for h in range(HPC):
            val = slopes[h] * (tg - np.float32(pos[b]))
            val[tg >= sl[b]] = NEG
            bias[:, b, h, :] = val

    hTf = np.ascontiguousarray(
        hidden.T.reshape(40, 128, B).transpose(1, 0, 2).reshape(128, 40 * B)
    )

    return dict(
        hT=hTf,
        qkvw=qkvw,
        ow=np.ascontiguousarray(o_w[es, :]),
        kt=kt,
        vt=vt,
        bias=np.ascontiguousarray(bias.reshape(128, B * HPC * 16)),
    )


def kernel(**inputs):
    global LAST_RESULTS
    hidden = np.asarray(inputs["hidden_states"], np.float32)
    qkv_w = np.asarray(inputs["qkv_weight"], np.float32)
    o_w = np.asarray(inputs["o_proj_weight"], np.float32)
    k_cache = np.asarray(inputs["k_cache"], np.float32)
    v_cache = np.asarray(inputs["v_cache"], np.float32)
    bt = np.asarray(inputs["block_tables"]).astype(np.int64)
    sl = np.asarray(inputs["sequence_lengths"]).astype(np.int64)

    pos = tuple(int(x) - 1 for x in sl)
    nch = tuple(int(math.ceil(int(x) / 128)) for x in sl)

    in_maps = [
        _prepare_core_inputs(c, hidden, qkv_w, o_w, k_cache, v_cache, bt, sl, pos)
        for c in range(NCORES)
    ]

    key = (pos, nch)
    if key not in _PROGRAM_CACHE:
        _PROGRAM_CACHE[key] = _build_program(pos, nch)
    nc = _PROGRAM_CACHE[key]

    from concourse.bass_utils import run_bass_kernel_spmd

    res = run_bass_kernel_spmd(
        nc,
        in_maps,
        core_ids=list(range(NCORES)),
        trace=bool(os.environ.get("BASS_TRACE")),
    )
    LAST_RESULTS = res

    out = np.zeros((B, E), np.float64)
    for c in range(NCORES):
        r = np.asarray(res.results[c]["outT"])
        out += r.reshape(128, 40, B).transpose(2, 1, 0).reshape(B, E).astype(np.float64)
    return out.astype(np.float32)



# revision 22
# speedup vs baseline: 1.8673x; 1.8673x over previous
"""Paged KV-cache decode attention with ALiBi (Baichuan-style), fused
QKV + attention + output projection, tensor-parallel over heads across
8 Trainium2 NeuronCores.

v2 design (bf16, minimal tensor-instruction count, DMA-roofline bound):
  - All matmuls in bf16: fp32 matmuls cost 2x (LDWEIGHTS+MATMUL) pairs
    on HW and 4x stream cycles; bf16 also halves HBM traffic.
  - Every matmul streams the BIG operand (weights/KV cache) as rhs with
    a tiny (1-4 col) lhsT weight load, maximizing work per instruction:
      * QKV:    lhsT = hT chunk [128,4],  rhs = W chunk [128,480]
      * scores: lhsT = qT col  [128,1],  rhs = K^T     [128,<=512]
      * attnV:  lhsT = attnT   [128,4],  rhs = V 4-pair blk [128,512]
      * o_proj: lhsT = aoT     [128,4],  rhs = W_o     [128,512]
  - ALiBi bias + causal mask folded multiplicatively: attn = exp(s) *
    ebias, ebias = exp(slope*(t-pos)) * (t<sl), host-precomputed (<=1,
    no overflow). Normalizer via one DVE tensor_tensor_reduce per pair.
  - Softmax normalization folded into the psum->sbuf extraction copies
    via activation(Copy, scale=recip[pair]).
  - Transposes (q/k, attn, attn-out) batched on the PE via identity
    matmuls, 31 total.
"""

import math
import os
import sys
from contextlib import ExitStack

import numpy as np
import ml_dtypes

sys.path.insert(0, "/opt/trn_rl_repo")

BF16 = ml_dtypes.bfloat16

B = 4
E = 5120
H = 40
D = 128
BS = 16
NB = 512
MB = 128
S = MB * BS  # 2048
NCORES = 8
HPC = H // NCORES   # 5 heads per core
EPC = HPC * D       # 640
NP = HPC * B        # 20 (b,h) pairs per core; pair r = h*4 + b


def _alibi_slopes(num_heads):
    cp2 = 2 ** int(math.floor(math.log2(num_heads)))
    base = 2.0 ** (-(2.0 ** (-(math.log2(cp2) - 3))))
    slopes = base ** np.arange(1, cp2 + 1, dtype=np.float64)
    if cp2 != num_heads:
        extra_base = 2.0 ** (-(2.0 ** (-(math.log2(2 * cp2) - 3))))
        n_rem = min(cp2, num_heads - cp2)
        extra = extra_base ** np.arange(1, 1 + 2 * n_rem, 2, dtype=np.float64)
        slopes = np.concatenate([slopes, extra])
    return slopes.astype(np.float64)


_PROGRAM_CACHE = {}
LAST_RESULTS = None  # BassKernelResults of the most recent run (for test.py)


def _build_program(sl):
    """SPMD Bass program; per-sequence lengths sl baked statically."""
    import concourse.bacc as bacc
    import concourse.bass as bass
    import concourse.tile as tile
    from concourse import masks, mybir

    f32 = mybir.dt.float32
    bf16 = mybir.dt.bfloat16
    Exp = mybir.ActivationFunctionType.Exp
    Copy = mybir.ActivationFunctionType.Copy

    pos = [s - 1 for s in sl]
    ncht = [(s + 127) // 128 for s in sl]     # 128-chunks per seq
    max_nch = max(ncht)

    nc = bacc.Bacc()

    hT = nc.declare_dram_parameter("hT", [128, 40 * B], bf16, isOutput=False)
    qkvw = nc.declare_dram_parameter("qkvw", [40, 128, 3 * EPC], bf16, isOutput=False)
    ow = nc.declare_dram_parameter("ow", [HPC, 128, E], bf16, isOutput=False)
    kt = nc.declare_dram_parameter("kt", [HPC, B, D, S], bf16, isOutput=False)
    vt = nc.declare_dram_parameter("vt", [HPC, 128, 16, B * D], bf16, isOutput=False)
    ebias = nc.declare_dram_parameter("ebias", [NP, S], bf16, isOutput=False)
    out = nc.declare_dram_parameter("out", [B, E], f32, isOutput=True)

    with tile.TileContext(nc) as tc, ExitStack() as ctx:
        consts = ctx.enter_context(tc.tile_pool(name="consts", bufs=1))
        wq = ctx.enter_context(tc.tile_pool(name="wq", bufs=3))
        wo = ctx.enter_context(tc.tile_pool(name="wo", bufs=2))
        kpool = ctx.enter_context(tc.tile_pool(name="kpool", bufs=1))
        vpool = ctx.enter_context(tc.tile_pool(name="vpool", bufs=2))

        # ---- constants / small inputs ----
        ident = consts.tile([32, 32], bf16)
        masks.make_identity(nc, ident[:])
        ones_col = consts.tile([128, 1], bf16)
        nc.vector.memset(ones_col[:], 1.0)
        hT_sb = consts.tile([128, 40 * B], bf16)
        nc.gpsimd.dma_start(out=hT_sb[:], in_=hT[:])

        qkT_sb = consts.tile([128, 2 * NP], bf16)   # q cols r, k cols 20+r
        attn_p = consts.tile([NP, S], bf16)         # exp(s)*ebias rows
        nc.vector.memset(attn_p[:], 0.0)
        attnT_sb = consts.tile([128, 16 * NP], bf16)
        rc4 = consts.tile([B, HPC], f32)            # 1/norm, [b, g]
        ao_nat = consts.tile([B, EPC], f32)         # attn-out rows (unnormalized)
        ao_bf = consts.tile([B, EPC], bf16)
        aoT_sb = consts.tile([128, NP], bf16)
        out_sb = consts.tile([B, E], f32)

        # K tiles all resident (exact per-seq size); DMAs fire immediately
        # and stream in parallel with the QKV phase.
        k_tiles = {}
        for g in range(HPC):
            for b in range(B):
                t = kpool.tile([128, sl[b]], bf16, tag=f"K{g}_{b}",
                               name=f"K_{g}_{b}")
                k_tiles[(g, b)] = t
                nc.gpsimd.dma_start(out=t[:], in_=kt[g, b, :, : sl[b]])

        # ---- Phase Q: fused QKV projection ----
        with tc.tile_pool(name="psq", bufs=4, space="PSUM") as psq:
            qkv_ps = [psq.tile([B, 480], f32, tag="ps", name=f"qkv{j}")
                      for j in range(4)]
            for kc in range(40):
                wt = wq.tile([128, 3 * EPC], bf16, tag="w")
                nc.gpsimd.dma_start(out=wt[:], in_=qkvw[kc])
                for j in range(4):
                    nc.tensor.matmul(
                        qkv_ps[j][:],
                        lhsT=hT_sb[:, kc * 4:(kc + 1) * 4],
                        rhs=wt[:, j * 480:(j + 1) * 480],
                        start=(kc == 0),
                        stop=(kc == 39),
                    )
            qkv_nat = consts.tile([B, 3 * EPC], bf16)
            for j in range(4):
                nc.scalar.copy(qkv_nat[:, j * 480:(j + 1) * 480], qkv_ps[j][:])

            # transpose q,k -> [128(d), 20] with col r = h*4+b
            qkT_ps = psq.tile([128, 2 * NP], bf16, tag="tp")
            for w in range(2):
                for h in range(HPC):
                    nc.tensor.matmul(
                        qkT_ps[:, w * NP + h * 4: w * NP + (h + 1) * 4],
                        lhsT=qkv_nat[:, w * EPC + h * D: w * EPC + (h + 1) * D],
                        rhs=ident[:B, :B],
                        is_transpose=True,
                    )
            nc.scalar.copy(qkT_sb[:], qkT_ps[:])
        v_sb = qkv_nat[:, 2 * EPC: 3 * EPC]  # natural v rows [4, 640]

        # ---- Phase S: scores + exp + ebias-mult (per pair) ----
        # Engine ops may only address partition base 0, so each pair works
        # in its own [1, S] tiles; rows are assembled into attn_p / rc4 by
        # tiny SBUF->SBUF DMAs (which may target any partition).
        rowpool = ctx.enter_context(tc.tile_pool(name="rowpool", bufs=2))
        with tc.tile_pool(name="pss", bufs=2, space="PSUM") as pss:
            for g in range(HPC):
                for b in range(B):
                    r = g * 4 + b
                    Ktile = k_tiles[(g, b)]
                    # scatter new-token k column (same partitions)
                    nc.vector.tensor_copy(
                        Ktile[:, pos[b]: pos[b] + 1], qkT_sb[:, NP + r: NP + r + 1]
                    )
                    s_ps = pss.tile([1, S], f32, tag="s")
                    nq = (sl[b] + 511) // 512
                    for qq in range(nq):
                        ncols = min(512, sl[b] - qq * 512)
                        nc.tensor.matmul(
                            s_ps[:, qq * 512: qq * 512 + ncols],
                            lhsT=qkT_sb[:, r: r + 1],
                            rhs=Ktile[:, qq * 512: qq * 512 + ncols],
                            start=True,
                            stop=True,
                        )
                    eb_t = rowpool.tile([1, S], bf16, tag="eb")
                    nc.gpsimd.dma_start(
                        out=eb_t[:, : sl[b]], in_=ebias[r: r + 1, : sl[b]]
                    )
                    ae_t = rowpool.tile([1, S], bf16, tag="ae")
                    nc.scalar.activation(
                        ae_t[:, : sl[b]], s_ps[:, : sl[b]], func=Exp
                    )
                    ap_t = rowpool.tile([1, S], bf16, tag="ap")
                    nc.vector.tensor_mul(
                        ap_t[:, : sl[b]], ae_t[:, : sl[b]], eb_t[:, : sl[b]]
                    )
                    # assemble rows at their pair offsets via DMA
                    nc.gpsimd.dma_start(
                        out=attn_p[r: r + 1, : sl[b]], in_=ap_t[:, : sl[b]]
                    )

        # ---- Phase T: transpose attn rows -> attnT [128(t%128), (c, r)],
        # and softmax normalizers via ones-column matmuls over attnT ----
        recip_sb = consts.tile([NP, 1], f32)
        with tc.tile_pool(name="pst", bufs=1, space="PSUM") as pst:
            attnT_ps = pst.tile([128, 16 * NP], bf16, tag="tp")
            for c in range(max_nch):
                nc.tensor.matmul(
                    attnT_ps[:, c * NP:(c + 1) * NP],
                    lhsT=attn_p[:, c * 128:(c + 1) * 128],
                    rhs=ident[:NP, :NP],
                    is_transpose=True,
                )
            nc.scalar.copy(
                attnT_sb[:, : max_nch * NP], attnT_ps[:, : max_nch * NP]
            )
            norm_ps = pst.tile([NP, 1], f32, tag="nrm")
            for c in range(max_nch):
                nc.tensor.matmul(
                    norm_ps[:],
                    lhsT=attnT_sb[:, c * NP:(c + 1) * NP],
                    rhs=ones_col[:],
                    start=(c == 0),
                    stop=(c == max_nch - 1),
                )
            nc.vector.reciprocal(recip_sb[:], norm_ps[:])
        for g in range(HPC):
            for b in range(B):
                r = g * 4 + b
                nc.gpsimd.dma_start(
                    out=rc4[b: b + 1, g: g + 1], in_=recip_sb[r: r + 1, :]
                )

        # ---- Phase A: attn @ V per head-group (4 pairs per matmul) ----
        ng = max_nch
        with tc.tile_pool(name="psa", bufs=2, space="PSUM") as psa:
            for g in range(HPC):
                Vg = vpool.tile([128, ng * B * D], bf16, tag="V", name=f"V{g}")
                nc.gpsimd.dma_start(out=Vg[:], in_=vt[g, :, :ng, :])
                # scatter new-token v rows (cross-partition -> DMA)
                for b in range(B):
                    p = pos[b]
                    nc.gpsimd.dma_start(
                        out=Vg[p % 128: p % 128 + 1,
                               (p // 128) * B * D + b * D: (p // 128) * B * D + (b + 1) * D],
                        in_=v_sb[b: b + 1, g * D:(g + 1) * D],
                    )
                ao_ps = psa.tile([B, B * D], f32, tag="ao")
                for c in range(ng):
                    nc.tensor.matmul(
                        ao_ps[:],
                        lhsT=attnT_sb[:, c * NP + g * 4: c * NP + (g + 1) * 4],
                        rhs=Vg[:, c * B * D:(c + 1) * B * D],
                        start=(c == 0),
                        stop=(c == ng - 1),
                    )
                # evict psum -> sbuf, then diag-extract rows via DMA
                ao_full = rowpool.tile([B, B * D], f32, tag="aof")
                nc.scalar.copy(ao_full[:], ao_ps[:])
                for b in range(B):
                    nc.gpsimd.dma_start(
                        out=ao_nat[b: b + 1, g * D:(g + 1) * D],
                        in_=ao_full[b: b + 1, b * D:(b + 1) * D],
                    )
            # normalize (per-partition scalar = 1/norm) + cast to bf16
            for g in range(HPC):
                nc.vector.tensor_scalar_mul(
                    ao_bf[:, g * D:(g + 1) * D],
                    ao_nat[:, g * D:(g + 1) * D],
                    rc4[:, g: g + 1],
                )

            # transpose attn-out -> aoT [128(d), 20] col h*4+b
            aoT_ps = psa.tile([128, NP], bf16, tag="aot")
            for h in range(HPC):
                nc.tensor.matmul(
                    aoT_ps[:, h * 4:(h + 1) * 4],
                    lhsT=ao_bf[:, h * D:(h + 1) * D],
                    rhs=ident[:B, :B],
                    is_transpose=True,
                )
            nc.scalar.copy(aoT_sb[:], aoT_ps[:])

        # ---- Phase O: output projection (psum waves of 8 + 2 col-blocks;
        # ow column-slices streamed per wave, no bytes loaded twice) ----
        with tc.tile_pool(name="pso", bufs=8, space="PSUM") as pso:
            for jw, js in enumerate((range(8), range(8, 10))):
                o_ps = {j: pso.tile([B, 512], f32, tag="o", name=f"o{j}")
                        for j in js}
                cs, ce = js[0] * 512, (js[-1] + 1) * 512
                for h in range(HPC):
                    owt = wo.tile([128, ce - cs], bf16, tag=f"ow{jw}",
                                  name=f"ow{jw}_{h}")
                    nc.gpsimd.dma_start(out=owt[:], in_=ow[h, :, cs:ce])
                    for j in js:
                        nc.tensor.matmul(
                            o_ps[j][:],
                            lhsT=aoT_sb[:, h * 4:(h + 1) * 4],
                            rhs=owt[:, j * 512 - cs: (j + 1) * 512 - cs],
                            start=(h == 0),
                            stop=(h == HPC - 1),
                        )
                for j in js:
                    nc.scalar.copy(out_sb[:, j * 512:(j + 1) * 512], o_ps[j][:])

        nc.gpsimd.dma_start(out=out[:], in_=out_sb[:])

    nc.compile()
    return nc


def _prepare_core_inputs(core, hidden_bf, qkv_w, o_w, k_cache, v_cache, bt, sl):
    hs = slice(core * HPC, (core + 1) * HPC)
    es = slice(core * EPC, (core + 1) * EPC)

    # qkvw: [40, 128, 1920] with cols [q | k | v], q pre-scaled
    wq = qkv_w[0][:, es] * np.float32(D ** -0.5)
    wk = qkv_w[1][:, es]
    wv = qkv_w[2][:, es]
    qkvw = np.concatenate([wq, wk, wv], axis=1).astype(BF16)  # [5120, 1920]
    qkvw = np.ascontiguousarray(qkvw.reshape(40, 128, 3 * EPC))

    ow_s = np.ascontiguousarray(
        o_w[es, :].astype(BF16).reshape(HPC, 128, E)
    )

    kg = k_cache[:, hs]  # [NB, HPC, BS, D]
    vg = v_cache[:, hs]
    kt = np.empty((HPC, B, D, S), BF16)
    vt = np.empty((HPC, 128, 16, B, D), BF16)
    for b in range(B):
        kk = kg[bt[b]].transpose(1, 0, 2, 3).reshape(HPC, S, D)  # [h, t, d]
        kt[:, b] = kk.transpose(0, 2, 1).astype(BF16)
        vv = vg[bt[b]].transpose(1, 0, 2, 3).reshape(HPC, 16, 128, D)
        vt[:, :, :, b, :] = vv.transpose(0, 2, 1, 3).astype(BF16)

    slopes = _alibi_slopes(H)[core * HPC:(core + 1) * HPC]
    t_idx = np.arange(S, dtype=np.float64)
    eb = np.zeros((HPC, B, S), np.float32)
    for h in range(HPC):
        for b in range(B):
            ex = np.minimum(slopes[h] * (t_idx - np.float64(sl[b] - 1)), 0.0)
            v = np.exp(ex).astype(np.float32)
            v[t_idx >= sl[b]] = 0.0
            eb[h, b] = v
    ebias = np.ascontiguousarray(eb.reshape(NP, S).astype(BF16))

    return dict(
        hT=hidden_bf,
        qkvw=qkvw,
        ow=ow_s,
        kt=np.ascontiguousarray(kt),
        vt=np.ascontiguousarray(vt.reshape(HPC, 128, 16, B * D)),
        ebias=ebias,
    )


def kernel(**inputs):
    global LAST_RESULTS
    hidden = np.asarray(inputs["hidden_states"], np.float32)
    qkv_w = np.asarray(inputs["qkv_weight"], np.float32)
    o_w = np.asarray(inputs["o_proj_weight"], np.float32)
    k_cache = np.asarray(inputs["k_cache"], np.float32)
    v_cache = np.asarray(inputs["v_cache"], np.float32)
    bt = np.asarray(inputs["block_tables"]).astype(np.int64)
    sl_arr = np.asarray(inputs["sequence_lengths"]).astype(np.int64)
    sl = tuple(int(x) for x in sl_arr)

    # hT[p, kc*4+b] = hidden[b, kc*128+p]
    hidden_bf = np.ascontiguousarray(
        hidden.T.reshape(40, 128, B).transpose(1, 0, 2).reshape(128, 40 * B)
    ).astype(BF16)

    in_maps = [
        _prepare_core_inputs(c, hidden_bf, qkv_w, o_w, k_cache, v_cache, bt, sl)
        for c in range(NCORES)
    ]

    if sl not in _PROGRAM_CACHE:
        _PROGRAM_CACHE[sl] = _build_program(sl)
    nc = _PROGRAM_CACHE[sl]

    from concourse.bass_utils import run_bass_kernel_spmd

    res = run_bass_kernel_spmd(
        nc,
        in_maps,
        core_ids=list(range(NCORES)),
        trace=bool(os.environ.get("BASS_TRACE")),
    )
    LAST_RESULTS = res

    acc = np.zeros((B, E), np.float64)
    for c in range(NCORES):
        acc += np.asarray(res.results[c]["out"]).astype(np.float64)
    return acc.astype(np.float32)


# revision 28
# speedup vs baseline: 2.2592x; 1.2099x over previous
"""Paged KV-cache decode attention with ALiBi (Baichuan-style), fused
QKV + attention + output projection, tensor-parallel over heads across
8 Trainium2 NeuronCores.

v2 design (bf16, minimal tensor-instruction count, DMA-roofline bound):
  - All matmuls in bf16: fp32 matmuls cost 2x (LDWEIGHTS+MATMUL) pairs
    on HW and 4x stream cycles; bf16 also halves HBM traffic.
  - Every matmul streams the BIG operand (weights/KV cache) as rhs with
    a tiny (1-4 col) lhsT weight load, maximizing work per instruction:
      * QKV:    lhsT = hT chunk [128,4],  rhs = W chunk [128,480]
      * scores: lhsT = qT col  [128,1],  rhs = K^T     [128,<=512]
      * attnV:  lhsT = attnT   [128,4],  rhs = V 4-pair blk [128,512]
      * o_proj: lhsT = aoT     [128,4],  rhs = W_o     [128,512]
  - ALiBi bias + causal mask folded multiplicatively: attn = exp(s) *
    ebias, ebias = exp(slope*(t-pos)) * (t<sl), host-precomputed (<=1,
    no overflow). Normalizer via one DVE tensor_tensor_reduce per pair.
  - Softmax normalization folded into the psum->sbuf extraction copies
    via activation(Copy, scale=recip[pair]).
  - Transposes (q/k, attn, attn-out) batched on the PE via identity
    matmuls, 31 total.
"""

import math
import os
import sys
from contextlib import ExitStack

import numpy as np
import ml_dtypes

sys.path.insert(0, "/opt/trn_rl_repo")

BF16 = ml_dtypes.bfloat16

B = 4
E = 5120
H = 40
D = 128
BS = 16
NB = 512
MB = 128
S = MB * BS  # 2048
NCORES = 8
HPC = H // NCORES   # 5 heads per core
EPC = HPC * D       # 640
NP = HPC * B        # 20 (b,h) pairs per core; pair r = h*4 + b


def _alibi_slopes(num_heads):
    cp2 = 2 ** int(math.floor(math.log2(num_heads)))
    base = 2.0 ** (-(2.0 ** (-(math.log2(cp2) - 3))))
    slopes = base ** np.arange(1, cp2 + 1, dtype=np.float64)
    if cp2 != num_heads:
        extra_base = 2.0 ** (-(2.0 ** (-(math.log2(2 * cp2) - 3))))
        n_rem = min(cp2, num_heads - cp2)
        extra = extra_base ** np.arange(1, 1 + 2 * n_rem, 2, dtype=np.float64)
        slopes = np.concatenate([slopes, extra])
    return slopes.astype(np.float64)


_PROGRAM_CACHE = {}
LAST_RESULTS = None  # BassKernelResults of the most recent run (for test.py)


def _build_program(sl):
    """SPMD Bass program; per-sequence lengths sl baked statically."""
    import concourse.bacc as bacc
    import concourse.bass as bass
    import concourse.tile as tile
    from concourse import masks, mybir

    f32 = mybir.dt.float32
    bf16 = mybir.dt.bfloat16
    Exp = mybir.ActivationFunctionType.Exp
    Copy = mybir.ActivationFunctionType.Copy

    pos = [s - 1 for s in sl]
    ncht = [(s + 127) // 128 for s in sl]     # 128-chunks per seq
    max_nch = max(ncht)

    nc = bacc.Bacc()

    hT = nc.declare_dram_parameter("hT", [128, 40 * B], bf16, isOutput=False)
    qkvw = nc.declare_dram_parameter("qkvw", [40, 128, 3 * EPC], bf16, isOutput=False)
    ow = nc.declare_dram_parameter("ow", [HPC, 128, E], bf16, isOutput=False)
    kt = nc.declare_dram_parameter("kt", [HPC, B, D, S], bf16, isOutput=False)
    vt = nc.declare_dram_parameter("vt", [HPC, 128, 16, B * D], bf16, isOutput=False)
    ebias = nc.declare_dram_parameter("ebias", [NP, S], bf16, isOutput=False)
    out = nc.declare_dram_parameter("out", [B, E], f32, isOutput=True)

    with tile.TileContext(nc) as tc, ExitStack() as ctx:
        consts = ctx.enter_context(tc.tile_pool(name="consts", bufs=1))
        wq = ctx.enter_context(tc.tile_pool(name="wq", bufs=3))
        wo = ctx.enter_context(tc.tile_pool(name="wo", bufs=2))
        kpool = ctx.enter_context(tc.tile_pool(name="kpool", bufs=1))
        vpool = ctx.enter_context(tc.tile_pool(name="vpool", bufs=3))

        # ---- constants / small inputs ----
        ident = consts.tile([32, 32], bf16)
        masks.make_identity(nc, ident[:])
        ones_col = consts.tile([128, 1], bf16)
        nc.vector.memset(ones_col[:], 1.0)
        hT_sb = consts.tile([128, 40 * B], bf16)
        nc.gpsimd.dma_start(out=hT_sb[:], in_=hT[:])

        qkT_sb = consts.tile([128, 2 * NP], bf16)   # q cols r, k cols 20+r
        attn_p = consts.tile([NP, S], bf16)         # exp(s)*ebias rows
        nc.vector.memset(attn_p[:], 0.0)
        attnT_sb = consts.tile([128, 16 * NP], bf16)
        rc4 = consts.tile([B, HPC], f32)            # 1/norm, [b, g]
        ao_nat = consts.tile([B, EPC], f32)         # attn-out rows (unnormalized)
        ao_bf = consts.tile([B, EPC], bf16)
        aoT_sb = consts.tile([128, NP], bf16)
        out_sb = consts.tile([B, E], f32)

        # K tiles all resident (exact per-seq size); DMAs fire immediately
        # on the scalar HWDGE ring, in parallel with the qkvw stream on the
        # sync HWDGE ring.
        k_tiles = {}
        for g in range(HPC):
            for b in range(B):
                t = kpool.tile([128, sl[b]], bf16, tag=f"K{g}_{b}",
                               name=f"K_{g}_{b}")
                k_tiles[(g, b)] = t
                nc.scalar.dma_start(out=t[:], in_=kt[g, b, :, : sl[b]])

        # V tiles: first 3 prefetched up front (no ring wait -> no deadlock),
        # remaining 2 issued inside the attnV loop once slots free up.
        ngv = max_nch
        v_tiles = []
        for g in range(HPC):
            vg = vpool.tile([128, ngv * B * D], bf16, tag="V", name=f"V{g}")
            v_tiles.append(vg)
            if g < 3:
                nc.scalar.dma_start(out=vg[:], in_=vt[g, :, :ngv, :])

        # ---- Phase Q: fused QKV projection ----
        with tc.tile_pool(name="psq", bufs=4, space="PSUM") as psq:
            qkv_ps = [psq.tile([B, 480], f32, tag="ps", name=f"qkv{j}")
                      for j in range(4)]
            for kc in range(40):
                wt = wq.tile([128, 3 * EPC], bf16, tag="w")
                nc.sync.dma_start(out=wt[:], in_=qkvw[kc])
                for j in range(4):
                    nc.tensor.matmul(
                        qkv_ps[j][:],
                        lhsT=hT_sb[:, kc * 4:(kc + 1) * 4],
                        rhs=wt[:, j * 480:(j + 1) * 480],
                        start=(kc == 0),
                        stop=(kc == 39),
                    )
            qkv_nat = consts.tile([B, 3 * EPC], bf16)
            for j in range(4):
                nc.scalar.copy(qkv_nat[:, j * 480:(j + 1) * 480], qkv_ps[j][:])

            # transpose q,k -> [128(d), 20] with col r = h*4+b
            qkT_ps = psq.tile([128, 2 * NP], bf16, tag="tp")
            for w in range(2):
                for h in range(HPC):
                    nc.tensor.matmul(
                        qkT_ps[:, w * NP + h * 4: w * NP + (h + 1) * 4],
                        lhsT=qkv_nat[:, w * EPC + h * D: w * EPC + (h + 1) * D],
                        rhs=ident[:B, :B],
                        is_transpose=True,
                    )
            nc.scalar.copy(qkT_sb[:], qkT_ps[:])
        v_sb = qkv_nat[:, 2 * EPC: 3 * EPC]  # natural v rows [4, 640]

        # ---- Phase S: scores + exp + ebias-mult (per pair) ----
        # Engine ops may only address partition base 0, so each pair works
        # in its own [1, S] tiles; rows are assembled into attn_p / rc4 by
        # tiny SBUF->SBUF DMAs (which may target any partition).
        rowpool = ctx.enter_context(tc.tile_pool(name="rowpool", bufs=2))
        with tc.tile_pool(name="pss", bufs=2, space="PSUM") as pss:
            for g in range(HPC):
                for b in range(B):
                    r = g * 4 + b
                    Ktile = k_tiles[(g, b)]
                    # scatter new-token k column (same partitions)
                    nc.vector.tensor_copy(
                        Ktile[:, pos[b]: pos[b] + 1], qkT_sb[:, NP + r: NP + r + 1]
                    )
                    s_ps = pss.tile([1, S], f32, tag="s")
                    nq = (sl[b] + 511) // 512
                    for qq in range(nq):
                        ncols = min(512, sl[b] - qq * 512)
                        nc.tensor.matmul(
                            s_ps[:, qq * 512: qq * 512 + ncols],
                            lhsT=qkT_sb[:, r: r + 1],
                            rhs=Ktile[:, qq * 512: qq * 512 + ncols],
                            start=True,
                            stop=True,
                        )
                    eb_t = rowpool.tile([1, S], bf16, tag="eb")
                    nc.gpsimd.dma_start(
                        out=eb_t[:, : sl[b]], in_=ebias[r: r + 1, : sl[b]]
                    )
                    ae_t = rowpool.tile([1, S], bf16, tag="ae")
                    nc.scalar.activation(
                        ae_t[:, : sl[b]], s_ps[:, : sl[b]], func=Exp
                    )
                    ap_t = rowpool.tile([1, S], bf16, tag="ap")
                    nc.vector.tensor_mul(
                        ap_t[:, : sl[b]], ae_t[:, : sl[b]], eb_t[:, : sl[b]]
                    )
                    # assemble rows at their pair offsets via DMA (sync ring
                    # is idle once the qkvw stream finishes)
                    nc.sync.dma_start(
                        out=attn_p[r: r + 1, : sl[b]], in_=ap_t[:, : sl[b]]
                    )

        # ---- Phase T: transpose attn rows -> attnT [128(t%128), (c, r)],
        # and softmax normalizers via ones-column matmuls over attnT ----
        recip_sb = consts.tile([NP, 1], f32)
        with tc.tile_pool(name="pst", bufs=1, space="PSUM") as pst:
            attnT_ps = pst.tile([128, 16 * NP], bf16, tag="tp")
            for c in range(max_nch):
                nc.tensor.matmul(
                    attnT_ps[:, c * NP:(c + 1) * NP],
                    lhsT=attn_p[:, c * 128:(c + 1) * 128],
                    rhs=ident[:NP, :NP],
                    is_transpose=True,
                )
            nc.scalar.copy(
                attnT_sb[:, : max_nch * NP], attnT_ps[:, : max_nch * NP]
            )
            norm_ps = pst.tile([NP, 1], f32, tag="nrm")
            for c in range(max_nch):
                nc.tensor.matmul(
                    norm_ps[:],
                    lhsT=attnT_sb[:, c * NP:(c + 1) * NP],
                    rhs=ones_col[:],
                    start=(c == 0),
                    stop=(c == max_nch - 1),
                )
            nc.vector.reciprocal(recip_sb[:], norm_ps[:])
        for g in range(HPC):
            for b in range(B):
                r = g * 4 + b
                nc.gpsimd.dma_start(
                    out=rc4[b: b + 1, g: g + 1], in_=recip_sb[r: r + 1, :]
                )

        # ---- Phase A: attn @ V per head-group (4 pairs per matmul) ----
        ng = max_nch
        with tc.tile_pool(name="psa", bufs=2, space="PSUM") as psa:
            for g in range(HPC):
                Vg = v_tiles[g]
                if g >= 3:
                    nc.scalar.dma_start(out=Vg[:], in_=vt[g, :, :ng, :])
                # scatter new-token v rows (cross-partition -> DMA)
                for b in range(B):
                    p = pos[b]
                    nc.gpsimd.dma_start(
                        out=Vg[p % 128: p % 128 + 1,
                               (p // 128) * B * D + b * D: (p // 128) * B * D + (b + 1) * D],
                        in_=v_sb[b: b + 1, g * D:(g + 1) * D],
                    )
                ao_ps = psa.tile([B, B * D], f32, tag="ao")
                for c in range(ng):
                    nc.tensor.matmul(
                        ao_ps[:],
                        lhsT=attnT_sb[:, c * NP + g * 4: c * NP + (g + 1) * 4],
                        rhs=Vg[:, c * B * D:(c + 1) * B * D],
                        start=(c == 0),
                        stop=(c == ng - 1),
                    )
                # evict psum -> sbuf, then diag-extract rows via DMA
                ao_full = rowpool.tile([B, B * D], f32, tag="aof")
                nc.scalar.copy(ao_full[:], ao_ps[:])
                for b in range(B):
                    nc.gpsimd.dma_start(
                        out=ao_nat[b: b + 1, g * D:(g + 1) * D],
                        in_=ao_full[b: b + 1, b * D:(b + 1) * D],
                    )
            # normalize (per-partition scalar = 1/norm) + cast to bf16
            for g in range(HPC):
                nc.vector.tensor_scalar_mul(
                    ao_bf[:, g * D:(g + 1) * D],
                    ao_nat[:, g * D:(g + 1) * D],
                    rc4[:, g: g + 1],
                )

            # transpose attn-out -> aoT [128(d), 20] col h*4+b
            aoT_ps = psa.tile([128, NP], bf16, tag="aot")
            for h in range(HPC):
                nc.tensor.matmul(
                    aoT_ps[:, h * 4:(h + 1) * 4],
                    lhsT=ao_bf[:, h * D:(h + 1) * D],
                    rhs=ident[:B, :B],
                    is_transpose=True,
                )
            nc.scalar.copy(aoT_sb[:], aoT_ps[:])

        # ---- Phase O: output projection (psum waves of 8 + 2 col-blocks;
        # ow column-slices streamed per wave, no bytes loaded twice) ----
        with tc.tile_pool(name="pso", bufs=8, space="PSUM") as pso:
            for jw, js in enumerate((range(8), range(8, 10))):
                o_ps = {j: pso.tile([B, 512], f32, tag="o", name=f"o{j}")
                        for j in js}
                cs, ce = js[0] * 512, (js[-1] + 1) * 512
                for h in range(HPC):
                    owt = wo.tile([128, ce - cs], bf16, tag=f"ow{jw}",
                                  name=f"ow{jw}_{h}")
                    nc.sync.dma_start(out=owt[:], in_=ow[h, :, cs:ce])
                    for j in js:
                        nc.tensor.matmul(
                            o_ps[j][:],
                            lhsT=aoT_sb[:, h * 4:(h + 1) * 4],
                            rhs=owt[:, j * 512 - cs: (j + 1) * 512 - cs],
                            start=(h == 0),
                            stop=(h == HPC - 1),
                        )
                for j in js:
                    nc.scalar.copy(out_sb[:, j * 512:(j + 1) * 512], o_ps[j][:])

        nc.gpsimd.dma_start(out=out[:], in_=out_sb[:])

    nc.compile()
    return nc


def _prepare_core_inputs(core, hidden_bf, qkv_w, o_w, k_cache, v_cache, bt, sl):
    hs = slice(core * HPC, (core + 1) * HPC)
    es = slice(core * EPC, (core + 1) * EPC)

    # qkvw: [40, 128, 1920] with cols [q | k | v], q pre-scaled
    wq = qkv_w[0][:, es] * np.float32(D ** -0.5)
    wk = qkv_w[1][:, es]
    wv = qkv_w[2][:, es]
    qkvw = np.concatenate([wq, wk, wv], axis=1).astype(BF16)  # [5120, 1920]
    qkvw = np.ascontiguousarray(qkvw.reshape(40, 128, 3 * EPC))

    ow_s = np.ascontiguousarray(
        o_w[es, :].astype(BF16).reshape(HPC, 128, E)
    )

    kg = k_cache[:, hs]  # [NB, HPC, BS, D]
    vg = v_cache[:, hs]
    kt = np.empty((HPC, B, D, S), BF16)
    vt = np.empty((HPC, 128, 16, B, D), BF16)
    for b in range(B):
        kk = kg[bt[b]].transpose(1, 0, 2, 3).reshape(HPC, S, D)  # [h, t, d]
        kt[:, b] = kk.transpose(0, 2, 1).astype(BF16)
        vv = vg[bt[b]].transpose(1, 0, 2, 3).reshape(HPC, 16, 128, D)
        vt[:, :, :, b, :] = vv.transpose(0, 2, 1, 3).astype(BF16)

    slopes = _alibi_slopes(H)[core * HPC:(core + 1) * HPC]
    t_idx = np.arange(S, dtype=np.float64)
    eb = np.zeros((HPC, B, S), np.float32)
    for h in range(HPC):
        for b in range(B):
            ex = np.minimum(slopes[h] * (t_idx - np.float64(sl[b] - 1)), 0.0)
            v = np.exp(ex).astype(np.float32)
            v[t_idx >= sl[b]] = 0.0
            eb[h, b] = v
    ebias = np.ascontiguousarray(eb.reshape(NP, S).astype(BF16))

    return dict(
        hT=hidden_bf,
        qkvw=qkvw,
        ow=ow_s,
        kt=np.ascontiguousarray(kt),
        vt=np.ascontiguousarray(vt.reshape(HPC, 128, 16, B * D)),
        ebias=ebias,
    )


def kernel(**inputs):
    global LAST_RESULTS
    hidden = np.asarray(inputs["hidden_states"], np.float32)
    qkv_w = np.asarray(inputs["qkv_weight"], np.float32)
    o_w = np.asarray(inputs["o_proj_weight"], np.float32)
    k_cache = np.asarray(inputs["k_cache"], np.float32)
    v_cache = np.asarray(inputs["v_cache"], np.float32)
    bt = np.asarray(inputs["block_tables"]).astype(np.int64)
    sl_arr = np.asarray(inputs["sequence_lengths"]).astype(np.int64)
    sl = tuple(int(x) for x in sl_arr)

    # hT[p, kc*4+b] = hidden[b, kc*128+p]
    hidden_bf = np.ascontiguousarray(
        hidden.T.reshape(40, 128, B).transpose(1, 0, 2).reshape(128, 40 * B)
    ).astype(BF16)

    in_maps = [
        _prepare_core_inputs(c, hidden_bf, qkv_w, o_w, k_cache, v_cache, bt, sl)
        for c in range(NCORES)
    ]

    if sl not in _PROGRAM_CACHE:
        _PROGRAM_CACHE[sl] = _build_program(sl)
    nc = _PROGRAM_CACHE[sl]

    from concourse.bass_utils import run_bass_kernel_spmd

    res = run_bass_kernel_spmd(
        nc,
        in_maps,
        core_ids=list(range(NCORES)),
        trace=bool(os.environ.get("BASS_TRACE")),
    )
    LAST_RESULTS = res

    acc = np.zeros((B, E), np.float64)
    for c in range(NCORES):
        acc += np.asarray(res.results[c]["out"]).astype(np.float64)
    return acc.astype(np.float32)


# revision 42
# speedup vs baseline: 2.4415x; 1.0807x over previous
"""Paged KV-cache decode attention with ALiBi (Baichuan-style), fused
QKV + attention + output projection, tensor-parallel over heads across
8 Trainium2 NeuronCores.

v2 design (bf16, minimal tensor-instruction count, DMA-roofline bound):
  - All matmuls in bf16: fp32 matmuls cost 2x (LDWEIGHTS+MATMUL) pairs
    on HW and 4x stream cycles; bf16 also halves HBM traffic.
  - Every matmul streams the BIG operand (weights/KV cache) as rhs with
    a tiny (1-4 col) lhsT weight load, maximizing work per instruction:
      * QKV:    lhsT = hT chunk [128,4],  rhs = W chunk [128,480]
      * scores: lhsT = qT col  [128,1],  rhs = K^T     [128,<=512]
      * attnV:  lhsT = attnT   [128,4],  rhs = V 4-pair blk [128,512]
      * o_proj: lhsT = aoT     [128,4],  rhs = W_o     [128,512]
  - ALiBi bias + causal mask folded multiplicatively: attn = exp(s) *
    ebias, ebias = exp(slope*(t-pos)) * (t<sl), host-precomputed (<=1,
    no overflow). Normalizer via one DVE tensor_tensor_reduce per pair.
  - Softmax normalization folded into the psum->sbuf extraction copies
    via activation(Copy, scale=recip[pair]).
  - Transposes (q/k, attn, attn-out) batched on the PE via identity
    matmuls, 31 total.
"""

import math
import os
import sys
from contextlib import ExitStack

import numpy as np
import ml_dtypes

sys.path.insert(0, "/opt/trn_rl_repo")

BF16 = ml_dtypes.bfloat16
FP8 = ml_dtypes.float8_e4m3

B = 4
E = 5120
H = 40
D = 128
BS = 16
NB = 512
MB = 128
S = MB * BS  # 2048
NCORES = 8
HPC = H // NCORES   # 5 heads per core
EPC = HPC * D       # 640
NP = HPC * B        # 20 (b,h) pairs per core; pair r = h*4 + b


def _alibi_slopes(num_heads):
    cp2 = 2 ** int(math.floor(math.log2(num_heads)))
    base = 2.0 ** (-(2.0 ** (-(math.log2(cp2) - 3))))
    slopes = base ** np.arange(1, cp2 + 1, dtype=np.float64)
    if cp2 != num_heads:
        extra_base = 2.0 ** (-(2.0 ** (-(math.log2(2 * cp2) - 3))))
        n_rem = min(cp2, num_heads - cp2)
        extra = extra_base ** np.arange(1, 1 + 2 * n_rem, 2, dtype=np.float64)
        slopes = np.concatenate([slopes, extra])
    return slopes.astype(np.float64)


_PROGRAM_CACHE = {}
LAST_RESULTS = None  # BassKernelResults of the most recent run (for test.py)


def _build_program(sl):
    """SPMD Bass program; per-sequence lengths sl baked statically."""
    import concourse.bacc as bacc
    import concourse.bass as bass
    import concourse.tile as tile
    from concourse import masks, mybir

    f32 = mybir.dt.float32
    bf16 = mybir.dt.bfloat16
    fp8 = mybir.dt.float8e4
    Exp = mybir.ActivationFunctionType.Exp
    Copy = mybir.ActivationFunctionType.Copy
    sm_scale = 1.0 / math.sqrt(D)

    pos = [s - 1 for s in sl]
    ncht = [(s + 127) // 128 for s in sl]     # 128-chunks per seq
    max_nch = max(ncht)

    nc = bacc.Bacc()

    hT = nc.declare_dram_parameter("hT", [128, 40 * B], bf16, isOutput=False)
    qkvw = nc.declare_dram_parameter("qkvw", [40, 128, 3 * EPC], bf16, isOutput=False)
    ow = nc.declare_dram_parameter("ow", [HPC, 128, E], bf16, isOutput=False)
    kt = nc.declare_dram_parameter("kt", [HPC, B, D, S], bf16, isOutput=False)
    vt = nc.declare_dram_parameter("vt", [HPC, 128, 16, B * D], bf16, isOutput=False)
    ebias = nc.declare_dram_parameter("ebias", [NP, S], bf16, isOutput=False)
    out = nc.declare_dram_parameter("out", [B, E], f32, isOutput=True)

    with tile.TileContext(nc) as tc, ExitStack() as ctx:
        consts = ctx.enter_context(tc.tile_pool(name="consts", bufs=1))
        wq = ctx.enter_context(tc.tile_pool(name="wq", bufs=3))
        wo = ctx.enter_context(tc.tile_pool(name="wo", bufs=2))
        kpool = ctx.enter_context(tc.tile_pool(name="kpool", bufs=1))
        vpool = ctx.enter_context(tc.tile_pool(name="vpool", bufs=3))

        # ---- constants / small inputs ----
        ident = consts.tile([32, 32], bf16)
        masks.make_identity(nc, ident[:])
        ones_col = consts.tile([128, 1], bf16)
        nc.vector.memset(ones_col[:], 1.0)
        hT_sb = consts.tile([128, 40 * B], bf16)
        nc.gpsimd.dma_start(out=hT_sb[:], in_=hT[:])

        qkT_sb = consts.tile([128, 2 * NP], bf16)   # q cols r, k cols 20+r
        attn_p = consts.tile([NP, S], bf16)         # exp(s)*ebias rows
        nc.vector.memset(attn_p[:], 0.0)
        attnT_sb = consts.tile([128, 16 * NP], bf16)
        rc4 = consts.tile([B, HPC], f32)            # 1/norm, [b, g]
        ao_nat = consts.tile([B, EPC], f32)         # attn-out rows (unnormalized)
        ao_bf = consts.tile([B, EPC], bf16)
        aoT_sb = consts.tile([128, NP], bf16)
        out_sb = consts.tile([B, E], f32)

        # K tiles all resident (exact per-seq size); DMAs fire immediately
        # on the scalar HWDGE ring, in parallel with the qkvw stream on the
        # sync HWDGE ring.
        k_tiles = {}
        for g in range(HPC):
            for b in range(B):
                t = kpool.tile([128, sl[b]], bf16, tag=f"K{g}_{b}",
                               name=f"K_{g}_{b}")
                k_tiles[(g, b)] = t
                nc.scalar.dma_start(out=t[:], in_=kt[g, b, :, : sl[b]])

        # V tiles: first 3 prefetched up front (no ring wait -> no deadlock),
        # remaining 2 issued inside the attnV loop once slots free up.
        ngv = max_nch
        v_tiles = []
        for g in range(HPC):
            vg = vpool.tile([128, ngv * B * D], bf16, tag="V", name=f"V{g}")
            v_tiles.append(vg)
            if g < 3:
                nc.scalar.dma_start(out=vg[:], in_=vt[g, :, :ngv, :])

        # ---- Phase Q: fused QKV projection. The 4 psum quarters live at
        # PE column-groups {0,32,64,96} of ONE bank (tile_position col
        # tiling) so they stream concurrently through the array. ----
        with tc.tile_pool(name="psq", bufs=1, space="PSUM") as psq:
            qkv_ps = psq.tile([128, 480], f32, tag="ps")
            for kc in range(40):
                wt = wq.tile([128, 3 * EPC], bf16, tag="w")
                nc.sync.dma_start(out=wt[:], in_=qkvw[kc])
                for j in range(4):
                    nc.tensor.matmul(
                        qkv_ps[32 * j: 32 * j + 4, :],
                        lhsT=hT_sb[:, kc * 4:(kc + 1) * 4],
                        rhs=wt[:, j * 480:(j + 1) * 480],
                        start=(kc == 0),
                        stop=(kc == 39),
                        tile_position=(0, 32 * j),
                    )
            qkv_nat = consts.tile([B, 3 * EPC], bf16)
            for j in range(4):
                nc.scalar.copy(qkv_nat[:, j * 480:(j + 1) * 480],
                               qkv_ps[32 * j: 32 * j + 4, :])

            # transpose q,k -> [128(d), 20] with col r = h*4+b
            qkT_ps = psq.tile([128, 2 * NP], bf16, tag="tp")
            for w in range(2):
                for h in range(HPC):
                    nc.tensor.matmul(
                        qkT_ps[:, w * NP + h * 4: w * NP + (h + 1) * 4],
                        lhsT=qkv_nat[:, w * EPC + h * D: w * EPC + (h + 1) * D],
                        rhs=ident[:B, :B],
                        is_transpose=True,
                    )
            nc.scalar.copy(qkT_sb[:], qkT_ps[:])
        v_sb = qkv_nat[:, 2 * EPC: 3 * EPC]  # natural v rows [4, 640]

        # ---- Phase S: scores + exp + ebias-mult (per pair) ----
        # Engine ops may only address partition base 0, so each pair works
        # in its own [1, S] tiles; rows are assembled into attn_p / rc4 by
        # tiny SBUF->SBUF DMAs (which may target any partition).
        rowpool = ctx.enter_context(tc.tile_pool(name="rowpool", bufs=2))
        with tc.tile_pool(name="pss", bufs=2, space="PSUM") as pss:
            for g in range(HPC):
                for b in range(B):
                    r = g * 4 + b
                    Ktile = k_tiles[(g, b)]
                    # scatter new-token k column (same partitions)
                    nc.vector.tensor_copy(
                        Ktile[:, pos[b]: pos[b] + 1], qkT_sb[:, NP + r: NP + r + 1]
                    )
                    s_ps = pss.tile([1, S], f32, tag="s")
                    nq = (sl[b] + 511) // 512
                    for qq in range(nq):
                        ncols = min(512, sl[b] - qq * 512)
                        nc.tensor.matmul(
                            s_ps[:, qq * 512: qq * 512 + ncols],
                            lhsT=qkT_sb[:, r: r + 1],
                            rhs=Ktile[:, qq * 512: qq * 512 + ncols],
                            start=True,
                            stop=True,
                        )
                    eb_t = rowpool.tile([1, S], bf16, tag="eb")
                    nc.gpsimd.dma_start(
                        out=eb_t[:, : sl[b]], in_=ebias[r: r + 1, : sl[b]]
                    )
                    ae_t = rowpool.tile([1, S], bf16, tag="ae")
                    nc.scalar.activation(
                        ae_t[:, : sl[b]], s_ps[:, : sl[b]], func=Exp,
                        scale=sm_scale,
                    )
                    ap_t = rowpool.tile([1, S], bf16, tag="ap")
                    nc.vector.tensor_mul(
                        ap_t[:, : sl[b]], ae_t[:, : sl[b]], eb_t[:, : sl[b]]
                    )
                    # assemble rows at their pair offsets via DMA (sync ring
                    # is idle once the qkvw stream finishes)
                    nc.sync.dma_start(
                        out=attn_p[r: r + 1, : sl[b]], in_=ap_t[:, : sl[b]]
                    )

        # ---- Phase T: transpose attn rows -> attnT [128(t%128), (c, r)],
        # and softmax normalizers via ones-column matmuls over attnT ----
        recip_sb = consts.tile([NP, 1], f32)
        with tc.tile_pool(name="pst", bufs=1, space="PSUM") as pst:
            attnT_ps = pst.tile([128, 16 * NP], bf16, tag="tp")
            for c in range(max_nch):
                nc.tensor.matmul(
                    attnT_ps[:, c * NP:(c + 1) * NP],
                    lhsT=attn_p[:, c * 128:(c + 1) * 128],
                    rhs=ident[:NP, :NP],
                    is_transpose=True,
                )
            nc.scalar.copy(
                attnT_sb[:, : max_nch * NP], attnT_ps[:, : max_nch * NP]
            )
            norm_ps = pst.tile([NP, 1], f32, tag="nrm")
            for c in range(max_nch):
                nc.tensor.matmul(
                    norm_ps[:],
                    lhsT=attnT_sb[:, c * NP:(c + 1) * NP],
                    rhs=ones_col[:],
                    start=(c == 0),
                    stop=(c == max_nch - 1),
                )
            nc.vector.reciprocal(recip_sb[:], norm_ps[:])
        for g in range(HPC):
            for b in range(B):
                r = g * 4 + b
                nc.gpsimd.dma_start(
                    out=rc4[b: b + 1, g: g + 1], in_=recip_sb[r: r + 1, :]
                )

        # ---- Phase A: attn @ V per head-group (4 pairs per matmul) ----
        ng = max_nch
        with tc.tile_pool(name="psa", bufs=2, space="PSUM") as psa:
            for g in range(HPC):
                Vg = v_tiles[g]
                if g >= 3:
                    nc.scalar.dma_start(out=Vg[:], in_=vt[g, :, :ng, :])
                # scatter new-token v rows (cross-partition -> DMA)
                for b in range(B):
                    p = pos[b]
                    nc.gpsimd.dma_start(
                        out=Vg[p % 128: p % 128 + 1,
                               (p // 128) * B * D + b * D: (p // 128) * B * D + (b + 1) * D],
                        in_=v_sb[b: b + 1, g * D:(g + 1) * D],
                    )
                ao_ps = psa.tile([B, B * D], f32, tag="ao")
                for c in range(ng):
                    nc.tensor.matmul(
                        ao_ps[:],
                        lhsT=attnT_sb[:, c * NP + g * 4: c * NP + (g + 1) * 4],
                        rhs=Vg[:, c * B * D:(c + 1) * B * D],
                        start=(c == 0),
                        stop=(c == ng - 1),
                    )
                # evict psum -> sbuf, then diag-extract rows via DMA
                ao_full = rowpool.tile([B, B * D], f32, tag="aof")
                nc.scalar.copy(ao_full[:], ao_ps[:])
                for b in range(B):
                    nc.gpsimd.dma_start(
                        out=ao_nat[b: b + 1, g * D:(g + 1) * D],
                        in_=ao_full[b: b + 1, b * D:(b + 1) * D],
                    )
            # normalize (per-partition scalar = 1/norm) + cast to bf16
            for g in range(HPC):
                nc.vector.tensor_scalar_mul(
                    ao_bf[:, g * D:(g + 1) * D],
                    ao_nat[:, g * D:(g + 1) * D],
                    rc4[:, g: g + 1],
                )

            # transpose attn-out -> aoT [128(d), 20] col h*4+b
            aoT_ps = psa.tile([128, NP], bf16, tag="aot")
            for h in range(HPC):
                nc.tensor.matmul(
                    aoT_ps[:, h * 4:(h + 1) * 4],
                    lhsT=ao_bf[:, h * D:(h + 1) * D],
                    rhs=ident[:B, :B],
                    is_transpose=True,
                )
            nc.scalar.copy(aoT_sb[:], aoT_ps[:])

        # ---- Phase O: output projection. 10 col-blocks packed 4-wide via
        # tile_position across 3 psum banks - no waves, ow streamed once. ----
        with tc.tile_pool(name="pso", bufs=3, space="PSUM") as pso:
            o_ps = [pso.tile([128, 512], f32, tag="o", name=f"ob{i}")
                    for i in range(3)]
            for h in range(HPC):
                owt = wo.tile([128, E], bf16, tag="ow", name=f"ow_{h}")
                nc.sync.dma_start(out=owt[:], in_=ow[h])
                for j in range(10):
                    cg = 32 * (j % 4)
                    nc.tensor.matmul(
                        o_ps[j // 4][cg: cg + 4, :],
                        lhsT=aoT_sb[:, h * 4:(h + 1) * 4],
                        rhs=owt[:, j * 512:(j + 1) * 512],
                        start=(h == 0),
                        stop=(h == HPC - 1),
                        tile_position=(0, cg),
                    )
            for j in range(10):
                cg = 32 * (j % 4)
                nc.scalar.copy(out_sb[:, j * 512:(j + 1) * 512],
                               o_ps[j // 4][cg: cg + 4, :])

        nc.gpsimd.dma_start(out=out[:], in_=out_sb[:])

    nc.compile()
    return nc


def _prepare_core_inputs(core, hidden_bf, qkv_w, o_w, k_cache, v_cache, bt, sl):
    hs = slice(core * HPC, (core + 1) * HPC)
    es = slice(core * EPC, (core + 1) * EPC)

    # qkvw: [40, 128, 1920] bf16, cols [q | k | v]; sm_scale is applied
    # at the exp on-device rather than folded into wq.
    qkvw = np.concatenate(
        [qkv_w[0][:, es], qkv_w[1][:, es], qkv_w[2][:, es]], axis=1
    ).astype(BF16)
    qkvw = np.ascontiguousarray(qkvw.reshape(40, 128, 3 * EPC))

    ow_s = np.ascontiguousarray(
        o_w[es, :].astype(BF16).reshape(HPC, 128, E)
    )

    kg = k_cache[:, hs]  # [NB, HPC, BS, D]
    vg = v_cache[:, hs]
    kt = np.empty((HPC, B, D, S), BF16)
    vt = np.empty((HPC, 128, 16, B, D), BF16)
    for b in range(B):
        kk = kg[bt[b]].transpose(1, 0, 2, 3).reshape(HPC, S, D)  # [h, t, d]
        kt[:, b] = kk.transpose(0, 2, 1).astype(BF16)
        vv = vg[bt[b]].transpose(1, 0, 2, 3).reshape(HPC, 16, 128, D)
        vt[:, :, :, b, :] = vv.transpose(0, 2, 1, 3).astype(BF16)

    slopes = _alibi_slopes(H)[core * HPC:(core + 1) * HPC]
    t_idx = np.arange(S, dtype=np.float64)
    eb = np.zeros((HPC, B, S), np.float32)
    for h in range(HPC):
        for b in range(B):
            ex = np.minimum(slopes[h] * (t_idx - np.float64(sl[b] - 1)), 0.0)
            v = np.exp(ex).astype(np.float32)
            v[t_idx >= sl[b]] = 0.0
            eb[h, b] = v
    ebias = np.ascontiguousarray(eb.reshape(NP, S).astype(BF16))

    return dict(
        hT=hidden_bf,
        qkvw=qkvw,
        ow=ow_s,
        kt=np.ascontiguousarray(kt),
        vt=np.ascontiguousarray(vt.reshape(HPC, 128, 16, B * D)),
        ebias=ebias,
    )


def kernel(**inputs):
    global LAST_RESULTS
    hidden = np.asarray(inputs["hidden_states"], np.float32)
    qkv_w = np.asarray(inputs["qkv_weight"], np.float32)
    o_w = np.asarray(inputs["o_proj_weight"], np.float32)
    k_cache = np.asarray(inputs["k_cache"], np.float32)
    v_cache = np.asarray(inputs["v_cache"], np.float32)
    bt = np.asarray(inputs["block_tables"]).astype(np.int64)
    sl_arr = np.asarray(inputs["sequence_lengths"]).astype(np.int64)
    sl = tuple(int(x) for x in sl_arr)

    # hT[p, kc*4+b] = hidden[b, kc*128+p]
    hidden_bf = np.ascontiguousarray(
        hidden.T.reshape(40, 128, B).transpose(1, 0, 2).reshape(128, 40 * B)
    ).astype(BF16)

    in_maps = [
        _prepare_core_inputs(c, hidden_bf, qkv_w, o_w, k_cache, v_cache, bt, sl)
        for c in range(NCORES)
    ]

    if sl not in _PROGRAM_CACHE:
        _PROGRAM_CACHE[sl] = _build_program(sl)
    nc = _PROGRAM_CACHE[sl]

    from concourse.bass_utils import run_bass_kernel_spmd

    res = run_bass_kernel_spmd(
        nc,
        in_maps,
        core_ids=list(range(NCORES)),
        trace=bool(os.environ.get("BASS_TRACE")),
    )
    LAST_RESULTS = res

    acc = np.zeros((B, E), np.float64)
    for c in range(NCORES):
        acc += np.asarray(res.results[c]["out"]).astype(np.float64)
    return acc.astype(np.float32)
